# revision 10
# baseline (speedup 1.0000x reference)
"""Trainium2 Bass kernel: batched single-head causal attention.

Problem: x [8, 2048, 1024] f32; Wq/Wk/Wv [64, 1024] f32.
  Q = x @ Wq.T; K = x @ Wk.T; V = x @ Wv.T            (per batch)
  out = softmax(mask(Q K^T / sqrt(1024))) @ V          -> [8, 2048, 64]

Sharding: data-parallel over batch B=8 across the 8 NeuronCores (one batch
element per core); the small weights are replicated.

Per-core algorithm (T=2048, C=1024, H=64), all matmul operands bf16
(accumulation in fp32 PSUM; verified rel-err ~1e-3 vs the fp32 reference,
comfortably under the 2e-2 gate):
  - x is cast to bf16 on host and transposed DRAM->SBUF by the DMA XBAR
    engine (InstDmaTransposeAnt, 16x128 tiles) into xT [128, 8k, T],
    sliced per (C-chunk, tq-block) so projections can start early.  This
    removes all PE transposes of x and their PSUM->SBUF copies.
  - Projections in transposed layout: QT [64, T] (scaled by 1/32, folded
    into Wq on host) and stacked KVT [128, T] (KT rows 0:64, VT 64:128).
  - V re-transposed to natural V_aug [T, 65] (PE transpose w/ bf16
    identity) with a ones column so attention row-sums fall out of the
    attn @ V matmul.
  - Scores computed TRANSPOSED: sT[tk, tq] = K Q^T chunks [128, 512];
    fully-masked blocks skipped; softmax max-subtraction skipped (|s| <~
    1.5); exp on ACT straight out of PSUM (bf16 out); early (off-diagonal)
    chunks are issued as row-packed pairs (PE rows 0:64 / 64:128 via
    base-64 SBUF duplicates of QT/KT) sharing one exp instruction over a
    2-bank PSUM pair tile; diagonal chunks are column-trimmed and
    multiplied by precomputed 0/1 bf16 masks.
  - outT_aug [65, 512] = V_aug^T @ expT accumulated over tk chunks; final
    PE transpose back to natural [128, 65] chunks (fp32); DVE reciprocal +
    tensor_scalar multiply normalizes; DMA out.
"""

import numpy as np

import concourse.bass as bass
import concourse.mybir as mybir
import concourse.tile as tile
from concourse import bacc
from concourse.bass_utils import run_bass_kernel_spmd

B = 8
T = 2048
C = 1024
H = 64
P = 128
NT = T // P   # 16 row chunks
NCH = C // P  # 8 contraction chunks
NB = 4        # tq blocks
BQ = 512      # tq block size
F32 = mybir.dt.float32
BF16 = mybir.dt.bfloat16

# One exp instruction per row-packed early score pair (2-bank PSUM tiles).
EXP_PAIR = True


def declare_io(nc):
    """DRAM tensors; shared by kernel() and the looped timing harness."""
    return {
        "x": nc.dram_tensor("x", [T, C], BF16, kind="ExternalInput").ap(),
        "w": nc.dram_tensor("w", [C, 192], BF16, kind="ExternalInput").ap(),
        "masks": nc.dram_tensor("masks", [P, 4, BQ], BF16, kind="ExternalInput").ap(),
        "identb": nc.dram_tensor("identb", [P, P], BF16, kind="ExternalInput").ap(),
        "ident": nc.dram_tensor("ident", [P, P], F32, kind="ExternalInput").ap(),
        "out": nc.dram_tensor("out", [T, H], F32, kind="ExternalOutput").ap(),
    }


def build_nc():
    nc = bacc.Bacc("TRN2", target_bir_lowering=False)
    io = declare_io(nc)
    with tile.TileContext(nc) as tc:
        _emit(nc, tc, io)
    nc.compile()
    return nc


def _emit(nc, tc, io):
    import contextlib

    x_d, w_d, m_d = io["x"], io["w"], io["masks"]
    ib_d, i_d, o_d = io["identb"], io["ident"], io["out"]

    ctx = contextlib.ExitStack()
    with ctx:
        consts = ctx.enter_context(tc.tile_pool(name="consts", bufs=1))
        persist = ctx.enter_context(tc.tile_pool(name="persist", bufs=1))
        expp = ctx.enter_context(tc.tile_pool(name="expp", bufs=4))
        oaugp = ctx.enter_context(tc.tile_pool(name="oaugp", bufs=2))
        outp = ctx.enter_context(tc.tile_pool(name="outp", bufs=4))
        recp = ctx.enter_context(tc.tile_pool(name="recp", bufs=4))
        # PSUM: psP 3 banks + psC 2x2 banks + psT 1 bank = 8 banks exactly
        psP = ctx.enter_context(tc.tile_pool(name="psP", bufs=3, space="PSUM"))
        psC = ctx.enter_context(tc.tile_pool(name="psC", bufs=2, space="PSUM"))
        psT = ctx.enter_context(tc.tile_pool(name="psT", bufs=1, space="PSUM"))

        # ---- constants ----
        # identities ride the scalar-engine HWDGE queue (the sync queue is
        # reserved for the x transposes); w/masks on SWDGE (gpsimd) so they
        # don't delay the x transposes either.
        identb_sb = consts.tile([P, P], BF16, tag="identb")
        nc.scalar.dma_start(out=identb_sb, in_=ib_d)
        ident_sb = consts.tile([P, P], F32, tag="ident")
        nc.scalar.dma_start(out=ident_sb, in_=i_d)
        w_sb = consts.tile([P, NCH, 192], BF16, tag="w")
        nc.gpsimd.dma_start(out=w_sb, in_=w_d.rearrange("(k p) m -> p k m", p=P))
        masks_sb = consts.tile([P, 4, BQ], BF16, tag="masks")
        nc.gpsimd.dma_start(out=masks_sb, in_=m_d)

        # ---- persistent tiles ----
        xT = persist.tile([P, NCH, T], BF16, tag="xT")   # xT[p,k,t] = x[t, k*128+p]
        qt = persist.tile([64, T], BF16, tag="qt")       # QT (pre-scaled by 1/32)
        kvt = persist.tile([P, T], BF16, tag="kvt")      # rows 0:64 KT, 64:128 VT
        vaug = persist.tile([P, NT, H + 1], BF16, tag="vaug")  # V chunks + ones col
        # Base-64 duplicates of QT/KT: K=64 matmuls stream at half rate
        # (moving fetch uses only 64 partitions), so early score chunks are
        # issued as row-packed pairs -- the odd member needs both operands in
        # partitions 64:128.  SBUF->SBUF DMA shifts partitions.
        qt64 = persist.tile([P, T], BF16, tag="qt64")    # rows 64:128 = QT
        kt64 = persist.tile([P, T], BF16, tag="kt64")    # rows 64:128 = KT

        nc.vector.memset(vaug[:, :, H : H + 1], 1.0)

        # ---- x DMA transposes: DRAM -> xT, sliced (tq-block, C-chunk) ----
        for tr in range(NB):
            for k in range(NCH):
                nc.sync.dma_start(
                    out=xT[:, k, tr * BQ : (tr + 1) * BQ],
                    in_=x_d[tr * BQ : (tr + 1) * BQ, k * P : (k + 1) * P],
                    transpose=True,
                )

        # lag pipeline of chunk-wise score -> exp/mask -> AV matmul
        pending = []

        def flush_av(limit):
            while len(pending) > limit:
                av_t, ex_ap, i_, last_ = pending.pop(0)
                nc.tensor.matmul(
                    av_t,
                    lhsT=vaug[:, i_, 0 : H + 1],
                    rhs=ex_ap,
                    start=(i_ == 0),
                    stop=last_,
                )

        def c_score(n, i, hi=False, sp=None, plane=None):
            """Score matmul for chunk i of block n.  hi=True issues it in PE
            rows 64:128 (reading the base-64 QT/KT duplicates) so it runs
            concurrently with the preceding hi=False chunk."""
            d = i - 4 * n
            off = 128 * d if d > 0 else 0
            if sp is None:
                spt = psC.tile([P, 2, BQ], F32, tag="psc")
                sp = spt[:, 0, :]
                dst = sp[:, off:BQ]
            else:
                dst = sp[:, plane, off:BQ]
            if hi:
                nc.tensor.matmul(
                    dst,
                    lhsT=kt64[64:128, i * P : (i + 1) * P],
                    rhs=qt64[64:128, n * BQ + off : (n + 1) * BQ],
                    start=True,
                    stop=True,
                    tile_position=(64, 0),
                )
            else:
                nc.tensor.matmul(
                    dst,
                    lhsT=kvt[0:64, i * P : (i + 1) * P],
                    rhs=qt[:, n * BQ + off : (n + 1) * BQ],
                    start=True,
                    stop=True,
                )
            return sp, off

        def c_chunk(av, n, i, nchunks):
            """Unpaired (diagonal) chunk: trimmed score -> exp -> mask -> AV."""
            sp, off = c_score(n, i)
            d = i - 4 * n
            ex = expp.tile([P, BQ], BF16, tag="ex")
            nc.scalar.activation(
                out=ex[:, off:BQ],
                in_=sp[:, off:BQ],
                func=mybir.ActivationFunctionType.Exp,
            )
            if d >= 0:
                # only columns [off, off+128) can be masked: for j >= off+128,
                # p + 128*d <= 127 + 128*d < j always holds
                nc.vector.tensor_mul(
                    ex[:, off : off + P],
                    ex[:, off : off + P],
                    masks_sb[:, d, off : off + P],
                )
            pending.append((av[0:65, off:BQ], ex[:, off:BQ], i, i == nchunks - 1))
            flush_av(1)

        def c_pair(av, n, i, nchunks):
            """Row-packed early score pair sharing one [P, 2, BQ] PSUM tile
            and a single exp instruction over both halves."""
            assert i + 1 < 4 * n, "pairs are for early (unmasked) chunks"
            if EXP_PAIR:
                sp = psC.tile([P, 2, BQ], F32, tag="psc")
                c_score(n, i, hi=False, sp=sp, plane=0)
                c_score(n, i + 1, hi=True, sp=sp, plane=1)
                ex = expp.tile([P, 2, BQ], BF16, tag="ex2")
                nc.scalar.activation(
                    out=ex, in_=sp, func=mybir.ActivationFunctionType.Exp
                )
                pending.append((av[0:65, :], ex[:, 0, :], i, False))
                pending.append(
                    (av[0:65, :], ex[:, 1, :], i + 1, i + 1 == nchunks - 1)
                )
            else:
                sp1, _ = c_score(n, i, hi=False)
                sp2, _ = c_score(n, i + 1, hi=True)
                for sp_, ii in ((sp1, i), (sp2, i + 1)):
                    ex = expp.tile([P, BQ], BF16, tag="ex")
                    nc.scalar.activation(
                        out=ex, in_=sp_, func=mybir.ActivationFunctionType.Exp
                    )
                    pending.append((av[0:65, :], ex, ii, ii == nchunks - 1))
            flush_av(1)

        for n in range(NB):
            nchunks = 4 * (n + 1)

            # ---- Q projection for tq block n ----
            q_ps = psP.tile([64, BQ], F32, tag="psp")
            for k in range(NCH):
                nc.tensor.matmul(
                    q_ps,
                    lhsT=w_sb[:, k, 0:64],
                    rhs=xT[:, k, n * BQ : (n + 1) * BQ],
                    start=(k == 0),
                    stop=(k == NCH - 1),
                )
            nc.scalar.copy(out=qt[:, n * BQ : (n + 1) * BQ], in_=q_ps)
            # base-64 duplicate of this block's QT (for row-packed pairs);
            # HWDGE on the scalar queue: the serial SWDGE queue would park it
            # behind the previous block's output store.
            nc.scalar.dma_start(
                out=qt64[64:128, n * BQ : (n + 1) * BQ],
                in_=qt[:, n * BQ : (n + 1) * BQ],
            )

            # ---- early chunks (pairs): depend only on OLD kvt/vaug ----
            av = psP.tile([65, BQ], F32, tag="psp")
            for i in range(0, 4 * n, 2):
                c_pair(av, n, i, nchunks)

            # ---- K|V projection for tq block n ----
            kv_ps = psP.tile([P, BQ], F32, tag="psp")
            for k in range(NCH):
                nc.tensor.matmul(
                    kv_ps,
                    lhsT=w_sb[:, k, 64:192],
                    rhs=xT[:, k, n * BQ : (n + 1) * BQ],
                    start=(k == 0),
                    stop=(k == NCH - 1),
                )
            # K half first (unblocks diagonal scores), then V half
            nc.vector.tensor_copy(
                out=kvt[0:64, n * BQ : (n + 1) * BQ], in_=kv_ps[0:64, :]
            )
            nc.vector.tensor_copy(
                out=kvt[64:128, n * BQ : (n + 1) * BQ], in_=kv_ps[64:128, :]
            )
            # base-64 duplicate of this block's KT (for later blocks' pairs)
            nc.scalar.dma_start(
                out=kt64[64:128, n * BQ : (n + 1) * BQ],
                in_=kvt[0:64, n * BQ : (n + 1) * BQ],
            )

            # ---- V natural chunks for this block (bf16 PE transposes) ----
            for j in range(4 * n, 4 * n + 4):
                vp = psT.tile([P, H], BF16, tag="pst")
                nc.tensor.transpose(
                    out=vp,
                    in_=kvt[64:128, j * P : (j + 1) * P],
                    identity=identb_sb[64:128, 64:128],
                )
                nc.vector.tensor_copy(out=vaug[:, j, 0:H], in_=vp)

            # ---- diagonal chunks ----
            for i in range(4 * n, nchunks):
                c_chunk(av, n, i, nchunks)
            flush_av(0)

            # ---- transpose back, normalize, store ----
            oa = oaugp.tile([65, BQ], F32, tag="oa")
            nc.vector.tensor_copy(out=oa, in_=av)
            tpt = psC.tile([P, 2, BQ], F32, tag="psc")
            tp = tpt[:, 0, 0:288].rearrange("p (q c) -> p q c", c=72)
            for q in range(4):
                nc.tensor.transpose(
                    out=tp[:, q, 0:65],
                    in_=oa[:, q * P : (q + 1) * P],
                    identity=ident_sb[0:65, 0:65],
                )
            r = recp.tile([P, 4], F32, tag="r")
            nc.vector.reciprocal(r, tp[:, :, 64])
            ot = outp.tile([P, 4, H], F32, tag="ot")
            for q in range(4):
                nc.vector.tensor_scalar_mul(
                    ot[:, q, :], tp[:, q, 0:64], r[:, q : q + 1]
                )
            nc.gpsimd.dma_start(
                out=o_d[n * BQ : (n + 1) * BQ, :].rearrange(
                    "(q p) h -> p q h", p=P
                ),
                in_=ot,
            )


def host_inputs(Wq, Wk, Wv):
    """Replicated per-core constant inputs from the raw weights."""
    bf = mybir.dt.np(BF16)
    scale = np.float32(1.0 / np.sqrt(np.float32(C)))
    w = np.empty((C, 192), dtype=np.float32)
    w[:, 0:64] = Wq.T * scale
    w[:, 64:128] = Wk.T
    w[:, 128:192] = Wv.T
    p = np.arange(P, dtype=np.int64)[:, None, None]
    d = np.arange(4, dtype=np.int64)[None, :, None]
    j = np.arange(BQ, dtype=np.int64)[None, None, :]
    masks = ((p + 128 * d) <= j).astype(np.float32)
    return (
        w.astype(bf),
        masks.astype(bf),
        np.eye(P, dtype=np.float32).astype(bf),
        np.eye(P, dtype=np.float32),
    )


def kernel(x, Wq, Wk, Wv):
    x = np.asarray(x, dtype=np.float32)
    Wq = np.asarray(Wq, dtype=np.float32)
    Wk = np.asarray(Wk, dtype=np.float32)
    Wv = np.asarray(Wv, dtype=np.float32)
    assert x.shape == (B, T, C), x.shape

    bf = mybir.dt.np(BF16)
    xb = np.ascontiguousarray(x.astype(bf))
    w, masks, identb, ident = host_inputs(Wq, Wk, Wv)
    nc = build_nc()
    in_maps = [
        {
            "x": np.ascontiguousarray(xb[b]),
            "w": w,
            "masks": masks,
            "identb": identb,
            "ident": ident,
        }
        for b in range(B)
    ]
    try:
        res = run_bass_kernel_spmd(nc, in_maps, core_ids=list(range(B)))
    except Exception:
        # transient device/mesh hiccups happen through the tunnel; one retry
        res = run_bass_kernel_spmd(nc, in_maps, core_ids=list(range(B)))
    return np.stack([res.results[b]["out"] for b in range(B)], axis=0)


# revision 33
# speedup vs baseline: 1.2467x; 1.2467x over previous
"""Trainium2 Bass kernel: batched single-head causal attention.

Problem: x [8, 2048, 1024] f32; Wq/Wk/Wv [64, 1024] f32.
  Q = x @ Wq.T; K = x @ Wk.T; V = x @ Wv.T            (per batch)
  out = softmax(mask(Q K^T / sqrt(1024))) @ V          -> [8, 2048, 64]

Sharding: data-parallel over batch B=8 across the 8 NeuronCores (one batch
element per core); the small weights are replicated.

Per-core algorithm (T=2048, C=1024, H=64), all matmul operands bf16
(accumulation in fp32 PSUM; verified rel-err ~1e-3 vs the fp32 reference,
comfortably under the 2e-2 gate):
  - x is cast to bf16 on host and transposed DRAM->SBUF by the DMA XBAR
    engine (InstDmaTransposeAnt, 16x128 tiles) into xT [128, 8k, T],
    sliced per (C-chunk, tq-block) so projections can start early.  This
    removes all PE transposes of x and their PSUM->SBUF copies.
  - Projections in transposed layout: QT [64, T] (scaled by 1/32, folded
    into Wq on host) and stacked KVT [128, T] (KT rows 0:64, VT 64:128).
  - V re-transposed to natural V_aug [T, 65] (PE transpose w/ bf16
    identity) with a ones column so attention row-sums fall out of the
    attn @ V matmul.
  - Scores computed TRANSPOSED: sT[tk, tq] = K Q^T chunks [128, 512];
    fully-masked blocks skipped; softmax max-subtraction skipped (|s| <~
    1.5); exp on ACT straight out of PSUM (bf16 out); early (off-diagonal)
    chunks are issued as row-packed pairs (PE rows 0:64 / 64:128 via
    base-64 SBUF duplicates of QT/KT) sharing one exp instruction over a
    2-bank PSUM pair tile; diagonal chunks are column-trimmed and
    multiplied by precomputed 0/1 bf16 masks.
  - outT_aug [65, 512] = V_aug^T @ expT accumulated over tk chunks; final
    PE transpose back to natural [128, 65] chunks (fp32); DVE reciprocal +
    tensor_scalar multiply normalizes; DMA out.
"""

import numpy as np

import concourse.bass as bass
import concourse.mybir as mybir
import concourse.tile as tile
from concourse import bacc
from concourse.bass_utils import run_bass_kernel_spmd

B = 8
T = 2048
C = 1024
H = 64
P = 128
NT = T // P   # 16 row chunks
NCH = C // P  # 8 contraction chunks
NB = 4        # tq blocks
BQ = 512      # tq block size
F32 = mybir.dt.float32
BF16 = mybir.dt.bfloat16

# One exp instruction per row-packed early score pair (2-bank PSUM tiles).
EXP_PAIR = True


def declare_io(nc):
    """DRAM tensors; shared by kernel() and the looped timing harness."""
    return {
        "x": nc.dram_tensor("x", [T, C], BF16, kind="ExternalInput").ap(),
        "wq": nc.dram_tensor("wq", [C, 64], BF16, kind="ExternalInput").ap(),
        "wkv": nc.dram_tensor("wkv", [C, 128], BF16, kind="ExternalInput").ap(),
        "masks": nc.dram_tensor("masks", [P, P], BF16, kind="ExternalInput").ap(),
        "identb": nc.dram_tensor("identb", [P, P], BF16, kind="ExternalInput").ap(),
        "ident": nc.dram_tensor("ident", [P, P], F32, kind="ExternalInput").ap(),
        "out": nc.dram_tensor("out", [T, H], F32, kind="ExternalOutput").ap(),
    }


def build_nc():
    nc = bacc.Bacc("TRN2", target_bir_lowering=False)
    io = declare_io(nc)
    with tile.TileContext(nc) as tc:
        _emit(nc, tc, io)
    nc.compile()
    return nc


def _emit(nc, tc, io):
    import contextlib

    x_d, wq_d, wkv_d, m_d = io["x"], io["wq"], io["wkv"], io["masks"]
    ib_d, i_d, o_d = io["identb"], io["ident"], io["out"]

    ctx = contextlib.ExitStack()
    with ctx:
        consts = ctx.enter_context(tc.tile_pool(name="consts", bufs=1))
        persist = ctx.enter_context(tc.tile_pool(name="persist", bufs=1))
        expp = ctx.enter_context(tc.tile_pool(name="expp", bufs=4))
        oaugp = ctx.enter_context(tc.tile_pool(name="oaugp", bufs=2))
        outp = ctx.enter_context(tc.tile_pool(name="outp", bufs=4))
        recp = ctx.enter_context(tc.tile_pool(name="recp", bufs=4))
        # PSUM: psP 3 banks + psC 2x2 banks + psT 1 bank = 8 banks exactly
        psP = ctx.enter_context(tc.tile_pool(name="psP", bufs=3, space="PSUM"))
        psC = ctx.enter_context(tc.tile_pool(name="psC", bufs=2, space="PSUM"))
        psT = ctx.enter_context(tc.tile_pool(name="psT", bufs=1, space="PSUM"))

        # ---- constants ----
        # All consts ride the scalar-engine HWDGE queue (low latency; the
        # SWDGE path has ~4-5us latency and the sync queue is reserved for
        # the x transposes).  HWDGE ring slots recycle with depth 8, so
        # emission order is chosen to keep the critical chain short:
        #   wq -> tr=0 x-transposes -> (wkv, identb, masks, ident) -> tr=1.
        wq_sb = consts.tile([P, NCH, 64], BF16, tag="wq")
        nc.scalar.dma_start(out=wq_sb, in_=wq_d.rearrange("(k p) m -> p k m", p=P))
        wkv_sb = consts.tile([P, NCH, P], BF16, tag="wkv")
        identb_sb = consts.tile([P, P], BF16, tag="identb")
        masks_sb = consts.tile([P, P], BF16, tag="masks")
        ident_sb = consts.tile([P, P], F32, tag="ident")

        # ---- persistent tiles ----
        xT = persist.tile([P, NCH, T], BF16, tag="xT")   # xT[p,k,t] = x[t, k*128+p]
        qt = persist.tile([64, T], BF16, tag="qt")       # QT (pre-scaled by 1/32)
        kvt = persist.tile([P, T], BF16, tag="kvt")      # rows 0:64 KT, 64:128 VT
        vaug = persist.tile([P, NT, H + 1], BF16, tag="vaug")  # V chunks + ones col
        # Base-64 duplicates of QT/KT: K=64 matmuls stream at half rate
        # (moving fetch uses only 64 partitions), so early score chunks are
        # issued as row-packed pairs -- the odd member needs both operands in
        # partitions 64:128.  SBUF->SBUF DMA shifts partitions.
        qt64 = persist.tile([P, T], BF16, tag="qt64")    # rows 64:128 = QT
        kt64 = persist.tile([P, T], BF16, tag="kt64")    # rows 64:128 = KT

        nc.vector.memset(vaug[:, :, H : H + 1], 1.0)

        # ---- x DMA transposes: DRAM -> xT, sliced (T-half, C-chunk) ----
        TT = T // 2

        def xpose(tr, k):
            nc.sync.dma_start(
                out=xT[:, k, tr * TT : (tr + 1) * TT],
                in_=x_d[tr * TT : (tr + 1) * TT, k * P : (k + 1) * P],
                transpose=True,
            )

        for k in range(NCH):
            xpose(0, k)
        nc.scalar.dma_start(
            out=wkv_sb, in_=wkv_d.rearrange("(k p) m -> p k m", p=P)
        )
        for k in range(NCH):
            xpose(1, k)
        nc.scalar.dma_start(out=identb_sb, in_=ib_d)
        nc.scalar.dma_start(out=masks_sb, in_=m_d)
        nc.scalar.dma_start(out=ident_sb, in_=i_d)

        # lag pipeline of chunk-wise score -> exp/mask -> AV matmul
        pending = []

        def flush_av(limit):
            while len(pending) > limit:
                av_t, ex_ap, i_, last_ = pending.pop(0)
                nc.tensor.matmul(
                    av_t,
                    lhsT=vaug[:, i_, 0 : H + 1],
                    rhs=ex_ap,
                    start=(i_ == 0),
                    stop=last_,
                )

        def c_score(n, i, hi=False, sp=None, plane=None):
            """Score matmul for chunk i of block n.  hi=True issues it in PE
            rows 64:128 (reading the base-64 QT/KT duplicates) so it runs
            concurrently with the preceding hi=False chunk."""
            d = i - 4 * n
            off = 128 * d if d > 0 else 0
            if sp is None:
                spt = psC.tile([P, 2, BQ], F32, tag="psc")
                sp = spt[:, 0, :]
                dst = sp[:, off:BQ]
            else:
                dst = sp[:, plane, off:BQ]
            if hi:
                nc.tensor.matmul(
                    dst,
                    lhsT=kt64[64:128, i * P : (i + 1) * P],
                    rhs=qt64[64:128, n * BQ + off : (n + 1) * BQ],
                    start=True,
                    stop=True,
                    tile_position=(64, 0),
                )
            else:
                nc.tensor.matmul(
                    dst,
                    lhsT=kvt[0:64, i * P : (i + 1) * P],
                    rhs=qt[:, n * BQ + off : (n + 1) * BQ],
                    start=True,
                    stop=True,
                )
            return sp, off

        def c_chunk(av, n, i, nchunks):
            """Unpaired (diagonal) chunk: trimmed score -> exp -> mask -> AV."""
            sp, off = c_score(n, i)
            d = i - 4 * n
            ex = expp.tile([P, BQ], BF16, tag="ex")
            nc.scalar.activation(
                out=ex[:, off:BQ],
                in_=sp[:, off:BQ],
                func=mybir.ActivationFunctionType.Exp,
            )
            if d >= 0:
                # only columns [off, off+128) can be masked: for j >= off+128,
                # p + 128*d <= 127 + 128*d < j always holds; within that
                # window the mask is the same lower-triangle for every d
                nc.vector.tensor_mul(
                    ex[:, off : off + P],
                    ex[:, off : off + P],
                    masks_sb,
                )
            pending.append((av[0:65, off:BQ], ex[:, off:BQ], i, i == nchunks - 1))
            flush_av(1)

        def c_pair(av, n, i, nchunks):
            """Row-packed early score pair sharing one [P, 2, BQ] PSUM tile
            and a single exp instruction over both halves."""
            assert i + 1 < 4 * n, "pairs are for early (unmasked) chunks"
            if EXP_PAIR:
                sp = psC.tile([P, 2, BQ], F32, tag="psc")
                c_score(n, i, hi=False, sp=sp, plane=0)
                c_score(n, i + 1, hi=True, sp=sp, plane=1)
                ex = expp.tile([P, 2, BQ], BF16, tag="ex2")
                nc.scalar.activation(
                    out=ex, in_=sp, func=mybir.ActivationFunctionType.Exp
                )
                pending.append((av[0:65, :], ex[:, 0, :], i, False))
                pending.append(
                    (av[0:65, :], ex[:, 1, :], i + 1, i + 1 == nchunks - 1)
                )
            else:
                sp1, _ = c_score(n, i, hi=False)
                sp2, _ = c_score(n, i + 1, hi=True)
                for sp_, ii in ((sp1, i), (sp2, i + 1)):
                    ex = expp.tile([P, BQ], BF16, tag="ex")
                    nc.scalar.activation(
                        out=ex, in_=sp_, func=mybir.ActivationFunctionType.Exp
                    )
                    pending.append((av[0:65, :], ex, ii, ii == nchunks - 1))
            flush_av(2)

        for n in range(NB):
            nchunks = 4 * (n + 1)

            # ---- Q projection for tq block n ----
            q_ps = psP.tile([64, BQ], F32, tag="psp")
            for k in range(NCH):
                nc.tensor.matmul(
                    q_ps,
                    lhsT=wq_sb[:, k, :],
                    rhs=xT[:, k, n * BQ : (n + 1) * BQ],
                    start=(k == 0),
                    stop=(k == NCH - 1),
                )
            nc.scalar.copy(out=qt[:, n * BQ : (n + 1) * BQ], in_=q_ps)
            # base-64 duplicate of this block's QT (for row-packed pairs).
            # MUST be SWDGE: an SBUF->SBUF DMA issued on HWDGE runs
            # concurrently with the x DMA-transposes there — a known HW
            # hazard (corrupts results; Tile only serializes the SWDGE path).
            nc.gpsimd.dma_start(
                out=qt64[64:128, n * BQ : (n + 1) * BQ],
                in_=qt[:, n * BQ : (n + 1) * BQ],
            )

            # ---- K|V projection for tq block n ----
            # Before the pairs: the pair-hi matmuls need the qt64 duplicate,
            # so the KV matmuls cover that DMA's latency.
            kv_ps = psP.tile([P, BQ], F32, tag="psp")
            for k in range(NCH):
                nc.tensor.matmul(
                    kv_ps,
                    lhsT=wkv_sb[:, k, :],
                    rhs=xT[:, k, n * BQ : (n + 1) * BQ],
                    start=(k == 0),
                    stop=(k == NCH - 1),
                )
            # K half first (unblocks diagonal scores), then V half
            nc.vector.tensor_copy(
                out=kvt[0:64, n * BQ : (n + 1) * BQ], in_=kv_ps[0:64, :]
            )
            nc.vector.tensor_copy(
                out=kvt[64:128, n * BQ : (n + 1) * BQ], in_=kv_ps[64:128, :]
            )
            # base-64 duplicate of this block's KT (for later blocks' pairs);
            # SWDGE for the same SBUF->SBUF hazard reason as qt64.
            nc.gpsimd.dma_start(
                out=kt64[64:128, n * BQ : (n + 1) * BQ],
                in_=kvt[0:64, n * BQ : (n + 1) * BQ],
            )

            # ---- early chunks (pairs): depend only on OLD kvt/vaug ----
            # AV lag 2: PE issues the next pair's scores before the previous
            # pair's AV matmuls, hiding the exp latency.
            av = psP.tile([65, BQ], F32, tag="psp")
            for i in range(0, 4 * n, 2):
                c_pair(av, n, i, nchunks)

            # ---- V natural chunks for this block (bf16 PE transposes) ----
            for j in range(4 * n, 4 * n + 4):
                vp = psT.tile([P, H], BF16, tag="pst")
                nc.tensor.transpose(
                    out=vp,
                    in_=kvt[64:128, j * P : (j + 1) * P],
                    identity=identb_sb[64:128, 64:128],
                )
                nc.vector.tensor_copy(out=vaug[:, j, 0:H], in_=vp)

            # ---- diagonal chunks ----
            for i in range(4 * n, nchunks):
                c_chunk(av, n, i, nchunks)
            flush_av(0)

            # ---- transpose back, normalize, store ----
            oa = oaugp.tile([65, BQ], F32, tag="oa")
            nc.vector.tensor_copy(out=oa, in_=av)
            tpt = psC.tile([P, 2, BQ], F32, tag="psc")
            tp = tpt[:, 0, 0:288].rearrange("p (q c) -> p q c", c=72)
            for q in range(4):
                nc.tensor.transpose(
                    out=tp[:, q, 0:65],
                    in_=oa[:, q * P : (q + 1) * P],
                    identity=ident_sb[0:65, 0:65],
                )
            r = recp.tile([P, 4], F32, tag="r")
            nc.vector.reciprocal(r, tp[:, :, 64])
            ot = outp.tile([P, 4, H], F32, tag="ot")
            for q in range(4):
                nc.vector.tensor_scalar_mul(
                    ot[:, q, :], tp[:, q, 0:64], r[:, q : q + 1]
                )
            # SBUF->DRAM store on the sync HWDGE queue (free after the x
            # transposes; keeps the serial SWDGE queue for the qt64/kt64
            # duplicates so they don't park behind these stores).
            nc.sync.dma_start(
                out=o_d[n * BQ : (n + 1) * BQ, :].rearrange(
                    "(q p) h -> p q h", p=P
                ),
                in_=ot,
            )


def host_inputs(Wq, Wk, Wv):
    """Replicated per-core constant inputs from the raw weights."""
    bf = mybir.dt.np(BF16)
    scale = np.float32(1.0 / np.sqrt(np.float32(C)))
    wq = np.ascontiguousarray(Wq.T * scale)
    wkv = np.empty((C, 128), dtype=np.float32)
    wkv[:, 0:64] = Wk.T
    wkv[:, 64:128] = Wv.T
    p = np.arange(P, dtype=np.int64)[:, None]
    j = np.arange(P, dtype=np.int64)[None, :]
    masks = (p <= j).astype(np.float32)
    return (
        wq.astype(bf),
        wkv.astype(bf),
        masks.astype(bf),
        np.eye(P, dtype=np.float32).astype(bf),
        np.eye(P, dtype=np.float32),
    )


def kernel(x, Wq, Wk, Wv):
    x = np.asarray(x, dtype=np.float32)
    Wq = np.asarray(Wq, dtype=np.float32)
    Wk = np.asarray(Wk, dtype=np.float32)
    Wv = np.asarray(Wv, dtype=np.float32)
    assert x.shape == (B, T, C), x.shape

    bf = mybir.dt.np(BF16)
    xb = np.ascontiguousarray(x.astype(bf))
    wq, wkv, masks, identb, ident = host_inputs(Wq, Wk, Wv)
    nc = build_nc()
    in_maps = [
        {
            "x": np.ascontiguousarray(xb[b]),
            "wq": wq,
            "wkv": wkv,
            "masks": masks,
            "identb": identb,
            "ident": ident,
        }
        for b in range(B)
    ]
    try:
        res = run_bass_kernel_spmd(nc, in_maps, core_ids=list(range(B)))
    except Exception:
        # transient device/mesh hiccups happen through the tunnel; one retry
        res = run_bass_kernel_spmd(nc, in_maps, core_ids=list(range(B)))
    return np.stack([res.results[b]["out"] for b in range(B)], axis=0)
